# revision 16
# baseline (speedup 1.0000x reference)
"""Trainium2 Bass kernel for nn_CausalGemAttention.

Reference computation (B=2, T=2048, C=1024, H=16, d=64):
    qkv = x @ w_attn + b_attn ; q,k,v = split(qkv)
    p = sign(sign(p_param)+0.5) * clamp(|p_param|, 1e-4, 1e3)
    vc = clip(|v + 5|, 1e-10); z = p*ln(vc); zmax = max_T(z); v' = exp(z - zmax)
    att = causal_softmax(q k^T / sqrt(d)); mean = att @ v'
    y = exp((zmax + ln(mean)) / p) - 5 ; out = y @ w_proj + b_proj

Sharding: 8 cores = 2 (batch) x 4 (head groups of 4 heads / 256 channels).
Each core computes qkv for its head group (contraction over full C), local
attention, and a partial projection (w_proj rows of its channels); host sums
the 4 partials per batch and adds b_proj.

Matmul operands are bf16 with fp32 PSUM accumulation.  v' is centered per
channel before the PV matmul (mean = num''/den + cmid, v'' = v' - cmid)
to keep bf16 rounding out of the softmax average.  When p == 1 the final
transform is y = ezp*mean - 5 without per-tile ln/exp.

Schedule (v2): single pool region, fully pipelined.
  - inputs stream per 512-column t-block; v matmuls start as soon as block 0
    lands; ln/max/min per block; zmax folded into one Exp (scale=p).
  - scores use the stacked two-head k tile as stationary with per-head
    zero-padded q as moving operand (no padded k buffer needed).
  - the attention phase is paced by ScalarE exp; remaining PE work (q/k for
    the second head pair, v transposes, projection tiles) is injected one
    item per softmax pair-slot so the PE never bursts ahead of ACT.
  - post-chains read PV PSUM directly (reciprocal + scalar_tensor_tensor),
    projection PSUM is drained on DVE, output DMA on the idle Sync queue.
"""

import sys
sys.path.insert(0, "/opt/trn_rl_repo")

import numpy as np
import ml_dtypes

import concourse.bacc as bacc
import concourse.tile as tile
from concourse import mybir
from concourse.bass_utils import run_bass_kernel_spmd

F32 = mybir.dt.float32
F32R = mybir.dt.float32r
F16 = mybir.dt.float16
BF16 = mybir.dt.bfloat16
AF = mybir.ActivationFunctionType
ALU = mybir.AluOpType
AX = mybir.AxisListType

B, T, C, H, D = 2, 2048, 1024, 16, 64
P = 128
CL = 256            # channels per core (4 heads x 64)
KC = C // P         # 8 contraction chunks for qkv
NQ = T // 512       # 4 query blocks of 512
NK = T // P         # 16 key tiles of 128
SHIFT = 5.0
P_MIN, P_MAX, V_MIN = 1e-4, 1e3, 1e-10
SM_SCALE = 1.0 / 8.0  # 1/sqrt(64)

_CACHE = {}


def _build(fast_p1):
    nc = bacc.Bacc("TRN2", target_bir_lowering=False, debug=False)

    xt_d = nc.dram_tensor("xt", [C, T], BF16, kind="ExternalInput")
    wq_d = nc.dram_tensor("wq", [C, CL], BF16, kind="ExternalInput")
    wk_d = nc.dram_tensor("wk", [C, CL], BF16, kind="ExternalInput")
    wv_d = nc.dram_tensor("wv", [C, CL], BF16, kind="ExternalInput")
    wp_d = nc.dram_tensor("wp", [CL, C], BF16, kind="ExternalInput")
    # bps = [bq(2) | bk(2) | bv5(2) | pp(2)] per chunk, fp32
    bps_d = nc.dram_tensor("bps", [P, 8], F32, kind="ExternalInput")
    # cst = [masks(4x512) | ident(64) | onesc(16)] packed along free dim
    cst_d = nc.dram_tensor("cst", [P, 2128], BF16, kind="ExternalInput")
    or_d = nc.dram_tensor("onesr", [1, 64], F32R, kind="ExternalInput")
    out_d = nc.dram_tensor("out_p", [T, C], F16, kind="ExternalOutput")

    with tile.TileContext(nc) as tc:
        with (
            tc.tile_pool(name="consts", bufs=1) as cp,
            tc.tile_pool(name="big", bufs=1) as bg,
            tc.tile_pool(name="att", bufs=5) as att,
            tc.tile_pool(name="small", bufs=3) as sm,
            tc.tile_pool(name="outp", bufs=3) as op_,
            tc.tile_pool(name="psS", bufs=2, space="PSUM") as psS,
            tc.tile_pool(name="psV", bufs=2, space="PSUM") as psV,
            tc.tile_pool(name="psX", bufs=2, space="PSUM") as psX,
        ):
            # ------------- static tiles -------------
            cst = cp.tile([P, 2128], BF16)
            onesr = cp.tile([1, 64], F32R)
            bps = cp.tile([P, 8], F32)
            xt_sb = bg.tile([P, KC, T], BF16)
            wq_sb = bg.tile([P, KC, CL], BF16)
            wk_sb = bg.tile([P, KC, CL], BF16)
            wv_sb = bg.tile([P, KC, CL], BF16)
            wp_sb = bg.tile([P, 2, C], BF16)
            qT = bg.tile([P, 2, T], BF16)     # q^T: [c%128, c//128, t]
            kT = bg.tile([P, 2, T], BF16)     # stacked: rows 0:64 head even,
            #                                   rows 64:128 head odd (per m)
            qTp = bg.tile([P, 4, T], BF16)    # per-head q, zero-padded to 128
            vnat = bg.tile([P, 4, NK, 65], BF16)  # [tk%128, head, tk//128, d|1]
            yT = bg.tile([P, 2, T], BF16)
            vT = bg.tile([P, 2, T], F32)      # |v+5| -> ln -> v'
            vpT = bg.tile([P, 2, T], BF16)    # centered v'' (bf16)

            ident = cst[:, 2048:2112]
            bq_sb = bps[:, 0:2]
            bk_sb = bps[:, 2:4]
            bv5_sb = bps[:, 4:6]
            pp_sb = bps[:, 6:8]

            # ------------- input DMA (ordered for pipelining) -------------
            nc.gpsimd.dma_start(bps[:], bps_d[:])
            nc.sync.dma_start(wv_sb[:], wv_d[:].rearrange("(a p) m -> p a m", p=P))
            # xt block 0 in two chunk-halves so v matmuls start asap
            nc.sync.dma_start(
                xt_sb[:, 0:4, 0:512],
                xt_d[0:512, 0:512].rearrange("(a p) m -> p a m", p=P))
            nc.sync.dma_start(
                xt_sb[:, 4:8, 0:512],
                xt_d[512:1024, 0:512].rearrange("(a p) m -> p a m", p=P))
            for b_ in range(1, NQ):
                bsl = slice(b_ * 512, (b_ + 1) * 512)
                nc.sync.dma_start(
                    xt_sb[:, :, bsl],
                    xt_d[:, bsl].rearrange("(a p) m -> p a m", p=P))
            nc.sync.dma_start(wq_sb[:], wq_d[:].rearrange("(a p) m -> p a m", p=P))
            nc.sync.dma_start(wk_sb[:], wk_d[:].rearrange("(a p) m -> p a m", p=P))
            nc.sync.dma_start(cst[:], cst_d[:])
            nc.sync.dma_start(onesr[:], or_d[:])
            nc.sync.dma_start(wp_sb[:], wp_d[:].rearrange("(c p) n -> p c n", p=P))
            for h in range(4):
                nc.sync.dma_start(vnat[:, h, :, 64], cst_d[:, 2112:2128])

            # qTp zero padding: even heads pad rows 64:128, odd pad 0:64
            nc.gpsimd.memset(qTp[64:128, 0, :], 0.0)
            nc.gpsimd.memset(qTp[0:64, 1, :], 0.0)
            nc.gpsimd.memset(qTp[64:128, 2, :], 0.0)
            nc.gpsimd.memset(qTp[0:64, 3, :], 0.0)

            # ------------- p transform (tiny) -------------
            sgn = cp.tile([P, 2], F32)
            ab = cp.tile([P, 2], F32)
            p_sb = cp.tile([P, 2], F32)
            # allcp packs [ip | zmaxp | cmid | ezp | ecp5] x 2 chunks
            allcp = cp.tile([P, 5, 2], F32)
            ip_sb = allcp[:, 0, :]
            zmaxp = allcp[:, 1, :]
            cmid = allcp[:, 2, :]
            ezp = allcp[:, 3, :]
            ecp5 = allcp[:, 4, :]
            nc.scalar.activation(sgn[:], pp_sb[:], AF.Sign)
            nc.vector.tensor_scalar_add(sgn[:], sgn[:], 0.5)
            nc.scalar.activation(sgn[:], sgn[:], AF.Sign)
            nc.scalar.activation(ab[:], pp_sb[:], AF.Abs)
            nc.vector.tensor_scalar(ab[:], ab[:], float(P_MIN), float(P_MAX),
                                    ALU.max, ALU.min)
            nc.vector.tensor_tensor(p_sb[:], sgn[:], ab[:], ALU.mult)
            nc.vector.reciprocal(ip_sb[:], p_sb[:])

            if not fast_p1:
                blmax = cp.tile([P, 2, NQ], F32)
                blmin = cp.tile([P, 2, NQ], F32)
            mxa = cp.tile([P, 2], F32)
            mna = cp.tile([P, 2], F32)
            negzmax = cp.tile([P, 2], F32)
            zmin_g = cp.tile([P, 2], F32)
            # allh: per-head [64,1] base-0 views of allcp, [64, const, head]
            allh = cp.tile([64, 5, 4], F32)
            iph = allh[:, 0, :]
            zmh = allh[:, 1, :]
            cmh = allh[:, 2, :]
            eph = allh[:, 3, :]
            ech = allh[:, 4, :]

            # ------------- helpers for phase A -------------
            def derived_consts():
                # from mxa/mna (max/min of ln vc over the chosen range):
                # z = p*lnv; zmax = max(p*maxln, p*minln), zmin = min(...)
                nc.vector.tensor_tensor(mxa[:], p_sb[:], mxa[:], ALU.mult)
                nc.vector.tensor_tensor(mna[:], p_sb[:], mna[:], ALU.mult)
                nc.vector.tensor_tensor(zmin_g[:], mxa[:], mna[:], ALU.min)
                nc.vector.tensor_tensor(negzmax[:], mxa[:], mna[:], ALU.max)
                nc.vector.tensor_scalar_mul(negzmax[:], negzmax[:], -1.0)
                # cmid = 0.5*(1 + exp(zmin - zmax))
                for m in range(2):
                    nc.scalar.activation(cmid[:, m:m + 1], zmin_g[:, m:m + 1],
                                         AF.Exp, bias=negzmax[:, m:m + 1])
                nc.vector.tensor_scalar(cmid[:], cmid[:], 1.0, 0.5,
                                        ALU.add, ALU.mult)
                # ezp = exp(zmax); ecp5 = ezp*cmid - 5
                nc.scalar.activation(ezp[:], negzmax[:], AF.Exp, scale=-1.0)
                nc.vector.scalar_tensor_tensor(ecp5[:], ezp[:], 0.0, cmid[:],
                                               ALU.bypass, ALU.mult)
                nc.vector.tensor_scalar_add(ecp5[:], ecp5[:], -SHIFT)
                # zmaxp = zmax * ip (general-p path)
                nc.vector.scalar_tensor_tensor(zmaxp[:], negzmax[:], -1.0,
                                               ip_sb[:], ALU.mult, ALU.mult)
                # per-head constants at partition base 0
                nc.sync.dma_start(allh[:, :, 0::2], allcp[0:64, :, :])
                nc.sync.dma_start(allh[:, :, 1::2], allcp[64:128, :, :])

            def v_exp_block(b_):
                # v' = exp(p*lnv - zmax); v'' = v' - cmid
                bsl = slice(b_ * 512, (b_ + 1) * 512)
                for m in range(2):
                    nc.scalar.activation(vT[:, m, bsl], vT[:, m, bsl], AF.Exp,
                                         scale=p_sb[:, m:m + 1],
                                         bias=negzmax[:, m:m + 1])
                    nc.vector.tensor_scalar_sub(vpT[:, m, bsl], vT[:, m, bsl],
                                                cmid[:, m:m + 1])

            def trp_group(h, half):
                # transpose v''^T [d, tk] -> vnat [tk, d], 8 k-tiles
                base, ch = 64 * (h % 2), h // 2
                trp = psX.tile([P, 512], BF16, tag="x", name="trp")
                for j in range(8):
                    kt = half * 8 + j
                    nc.tensor.transpose(
                        trp[:, j * 64:(j + 1) * 64],
                        vpT[base:base + 64, ch, kt * P:(kt + 1) * P],
                        cst[base:base + 64, 2048:2112])
                nc.vector.tensor_copy(
                    vnat[:, h, half * 8:(half + 1) * 8, 0:64],
                    trp[:].rearrange("p (a b) -> p a b", a=8))

            def qk_seq(kind, m, nt, on_act):
                wsb = wq_sb if kind == "q" else wk_sb
                dst = qT if kind == "q" else kT
                bsb = bq_sb if kind == "q" else bk_sb
                ps = psX.tile([P, 512], F32, tag="x", name="ev")
                for kc in range(KC):
                    nc.tensor.matmul(
                        ps[:], wsb[:, kc, m * P:(m + 1) * P],
                        xt_sb[:, kc, nt * 512:(nt + 1) * 512],
                        start=(kc == 0), stop=(kc == KC - 1))
                tsl = slice(nt * 512, (nt + 1) * 512)
                if on_act:
                    nc.scalar.activation(dst[:, m, tsl], ps[:], AF.Identity,
                                         bias=bsb[:, m:m + 1])
                else:
                    nc.vector.tensor_scalar_add(dst[:, m, tsl], ps[:],
                                                bsb[:, m:m + 1])

            def qpads(m):
                h0, h1 = 2 * m, 2 * m + 1
                nc.gpsimd.dma_start(qTp[0:64, h0, :], qT[0:64, m, :])
                nc.gpsimd.dma_start(qTp[64:128, h1, :], qT[64:128, m, :])

            # ------------- per-block v/q/k matmuls + streamed transform ------
            # For fast_p1, the stabilizer zmax/cmid is computed from block 0
            # only (algebraically exact for any per-channel constant; p == 1
            # keeps exp(z - zref) bounded near 1).  General p keeps the exact
            # global max to avoid overflow under p up to 1e3.
            for b_ in range(NQ):
                bsl = slice(b_ * 512, (b_ + 1) * 512)
                for m in range(2):
                    ps = psX.tile([P, 512], F32, tag="x", name="ev")
                    for kc in range(KC):
                        nc.tensor.matmul(
                            ps[:], wv_sb[:, kc, m * P:(m + 1) * P],
                            xt_sb[:, kc, bsl],
                            start=(kc == 0), stop=(kc == KC - 1))
                    nc.scalar.activation(vT[:, m, bsl], ps[:], AF.Abs,
                                         bias=bv5_sb[:, m:m + 1])
                for m in range(2):
                    nc.vector.tensor_scalar_max(vT[:, m, bsl], vT[:, m, bsl],
                                                float(V_MIN))
                for m in range(2):
                    nc.scalar.activation(vT[:, m, bsl], vT[:, m, bsl], AF.Ln)
                if fast_p1:
                    if b_ == 0:
                        nc.vector.tensor_reduce(mxa[:, 0:1], vT[:, 0, bsl],
                                                AX.X, op=ALU.max)
                        nc.vector.tensor_reduce(mxa[:, 1:2], vT[:, 1, bsl],
                                                AX.X, op=ALU.max)
                        nc.vector.tensor_reduce(mna[:, 0:1], vT[:, 0, bsl],
                                                AX.X, op=ALU.min)
                        nc.vector.tensor_reduce(mna[:, 1:2], vT[:, 1, bsl],
                                                AX.X, op=ALU.min)
                        derived_consts()
                    v_exp_block(b_)
                else:
                    for m in range(2):
                        nc.vector.tensor_reduce(blmax[:, m, b_:b_ + 1],
                                                vT[:, m, bsl], AX.X,
                                                op=ALU.max)
                        nc.vector.tensor_reduce(blmin[:, m, b_:b_ + 1],
                                                vT[:, m, bsl], AX.X,
                                                op=ALU.min)
                # q/k for head pair 0 on this t-block (only needs xt block b)
                qk_seq("q", 0, b_, True)
                qk_seq("k", 0, b_, True)
                nc.gpsimd.dma_start(qTp[0:64, 0, bsl], qT[0:64, 0, bsl])
                nc.gpsimd.dma_start(qTp[64:128, 1, bsl], qT[64:128, 0, bsl])
                if fast_p1 and b_ == 1:
                    trp_group(0, 0)
                    trp_group(1, 0)

            if not fast_p1:
                nc.vector.tensor_reduce(mxa[:, 0:1], blmax[:, 0, :], AX.X,
                                        op=ALU.max)
                nc.vector.tensor_reduce(mxa[:, 1:2], blmax[:, 1, :], AX.X,
                                        op=ALU.max)
                nc.vector.tensor_reduce(mna[:, 0:1], blmin[:, 0, :], AX.X,
                                        op=ALU.min)
                nc.vector.tensor_reduce(mna[:, 1:2], blmin[:, 1, :], AX.X,
                                        op=ALU.min)
                derived_consts()
                for b_ in range(NQ):
                    v_exp_block(b_)
                trp_group(0, 0)
                trp_group(1, 0)

            # ------------- attention -------------
            pending = []   # deferred post-chains (emitted mid next q-block)
            projq = []     # projection tiles ready to emit

            po_ref = {}

            def proj_unit(tq, nh):
                if nh == 0:
                    po_ref[tq] = op_.tile([P, C], F16, tag="po", name="po")
                po = po_ref[tq]
                pj = psX.tile([P, 512], F32, tag="x", name="pj")
                for c in range(2):
                    nc.tensor.matmul(
                        pj[:], yT[:, c, tq * P:(tq + 1) * P],
                        wp_sb[:, c, nh * 512:(nh + 1) * 512],
                        start=(c == 0), stop=(c == 1))
                nc.vector.tensor_copy(po[:, nh * 512:(nh + 1) * 512], pj[:])
                if nh == 1:
                    nc.sync.dma_start(out_d[tq * P:(tq + 1) * P, :], po[:])
                    del po_ref[tq]

            def post_chain(pv_t, h, qi_):
                # mean'' = num''/den ; y = ezp*mean'' + (ezp*cmid - 5)
                dcp = sm.tile([1, 512], F32, tag="dcp", name="dcp")
                nc.vector.tensor_copy(dcp[:], pv_t[64:65, :])
                rdf = sm.tile([1, 512], F32, tag="rdf", name="rdf")
                nc.vector.reciprocal_approx_fast(rdf[:], dcp[:])
                bcs = sm.tile([64, 512], F32, tag="bc", name="bcs")
                nc.gpsimd.partition_broadcast(bcs[:], rdf[:])
                yh = sm.tile([64, 512], BF16, tag="yh", name="yh")
                if fast_p1:
                    y1 = sm.tile([64, 512], F32, tag="y1", name="y1")
                    nc.vector.scalar_tensor_tensor(
                        y1[:], pv_t[0:64, :], eph[:, h:h + 1], bcs[:],
                        ALU.mult, ALU.mult)
                    nc.vector.tensor_scalar_add(yh[:], y1[:], ech[:, h:h + 1])
                else:
                    me = sm.tile([64, 512], F32, tag="y1", name="me")
                    nc.vector.tensor_tensor(me[:], pv_t[0:64, :], bcs[:],
                                            ALU.mult)
                    nc.vector.tensor_scalar_add(me[:], me[:], cmh[:, h:h + 1])
                    nc.scalar.activation(me[:], me[:], AF.Ln)
                    nc.scalar.activation(yh[:], me[:], AF.Exp,
                                         scale=iph[:, h:h + 1],
                                         bias=zmh[:, h:h + 1])
                    nc.vector.tensor_scalar_add(yh[:], yh[:], -SHIFT)
                base, ch = 64 * (h % 2), h // 2
                nc.gpsimd.dma_start(
                    yT[base:base + 64, ch, qi_ * 512:(qi_ + 1) * 512], yh[:])

            # PE work injected one item per softmax pair-slot during hp=0
            inject = []
            inject.append(lambda: trp_group(0, 1))
            inject.append(lambda: trp_group(1, 1))
            for nt in range(NQ):
                inject.append(lambda nt=nt: qk_seq("q", 1, nt, False))
            inject.append(lambda: qpads(1))
            for nt in range(NQ):
                inject.append(lambda nt=nt: qk_seq("k", 1, nt, False))
            inject.append(lambda: trp_group(2, 0))
            inject.append(lambda: trp_group(3, 0))
            # second head pair's late transposes go into hp=1's idle slots
            inject2 = [lambda: trp_group(2, 1), lambda: trp_group(3, 1)]

            def flush_pending():
                for fn in pending:
                    fn()
                pending.clear()

            def slot_work(hp):
                if hp == 0:
                    if inject:
                        inject.pop(0)()
                else:
                    if inject2:
                        inject2.pop(0)()
                    elif projq:
                        proj_unit(*projq.pop(0))

            def mk(pv_t, h, qi_, push_proj):
                def fn():
                    post_chain(pv_t, h, qi_)
                    if push_proj:
                        projq.extend((tq, nh) for tq in
                                     range(4 * qi_, 4 * qi_ + 4)
                                     for nh in range(2))
                return fn

            def scores_mm(s_t, h, hp, qi, a):
                for half in range(2):
                    kt = 2 * a + half
                    off = P * max(kt - 4 * qi, 0)
                    ksl = slice(kt * P, (kt + 1) * P)
                    qsub = slice(qi * 512 + off, (qi + 1) * 512)
                    nc.tensor.matmul(
                        s_t[:, half * 512 + off:(half + 1) * 512],
                        kT[:, hp, ksl], qTp[:, h, qsub],
                        start=True, stop=True, skip_group_check=True)

            def exp_mask(s_t, qi, a):
                j0 = 2 * a - 4 * qi
                off0 = P * max(j0, 0)
                pt = att.tile([P, 1024], BF16, tag="pT", name="pt")
                nc.scalar.activation(pt[:, off0:1024], s_t[:, off0:1024],
                                     AF.Exp, scale=SM_SCALE)
                if j0 >= 0:   # diagonal band: mask pair
                    nc.vector.tensor_mul(
                        pt[:, off0:1024], pt[:, off0:1024],
                        cst[:, j0 * 512 + off0:(j0 + 2) * 512])
                return pt

            def pv_mm(pv_t, h, qi, a, pt, last):
                for half in range(2):
                    kt = 2 * a + half
                    o_ = P * max(kt - 4 * qi, 0)
                    nc.tensor.matmul(
                        pv_t[:, o_:512], vnat[:, h, kt, :],
                        pt[:, half * 512 + o_:(half + 1) * 512],
                        start=(kt == 0), stop=(last and half == 1),
                        skip_group_check=True)

            for hp in range(2):
                h0, h1 = 2 * hp, 2 * hp + 1
                nq_pair = NQ - 1 if hp == 1 else NQ
                for qi in range(nq_pair):
                    npair = 2 * (qi + 1)       # kt pairs (kt = 2a, 2a+1)
                    pv = [psV.tile([65, 512], F32, tag="pv", name=f"pv{_i}")
                          for _i in range(2)]
                    prev = None
                    for a in range(npair):
                        ptile = []
                        s_ps = [psS.tile([P, 1024], F32, tag="s",
                                         name=f"s{_i}") for _i in range(2)]
                        for i, h in enumerate((h0, h1)):
                            scores_mm(s_ps[i], h, hp, qi, a)
                        slot_work(hp)
                        for i in range(2):
                            ptile.append(exp_mask(s_ps[i], qi, a))
                        if prev is not None:
                            pa_, pp0, pp1 = prev
                            for i, ppt in enumerate((pp0, pp1)):
                                pv_mm(pv[i], (h0, h1)[i], qi, pa_, ppt, False)
                        if a == 0 and pending:
                            flush_pending()
                        if hp == 1 and projq:
                            proj_unit(*projq.pop(0))
                        prev = (a, ptile[0], ptile[1])
                    pa_, pp0, pp1 = prev
                    for i, ppt in enumerate((pp0, pp1)):
                        pv_mm(pv[i], (h0, h1)[i], qi, pa_, ppt, True)
                    pending.append(mk(pv[0], h0, qi, False))
                    pending.append(mk(pv[1], h1, qi, hp == 1))

            # last q-block of the second head pair: process heads one after
            # the other so h2's post-chain overlaps h3's scores/PV on PE
            qi = NQ - 1
            npair = 2 * (qi + 1)
            pv = [psV.tile([65, 512], F32, tag="pv", name=f"pvl{_i}")
                  for _i in range(2)]
            for i, h in ((0, 2), (1, 3)):
                prevh = None
                for a in range(npair):
                    s_ = psS.tile([P, 1024], F32, tag="s", name="sl")
                    scores_mm(s_, h, 1, qi, a)
                    if i == 0 and a == 0 and pending:
                        flush_pending()
                    if projq:
                        proj_unit(*projq.pop(0))
                    pt = exp_mask(s_, qi, a)
                    if prevh is not None:
                        pv_mm(pv[i], h, qi, prevh[0], prevh[1], False)
                    prevh = (a, pt)
                pv_mm(pv[i], h, qi, prevh[0], prevh[1], True)
                post_chain(pv[i], h, qi)
            projq.extend((tq, nh) for tq in range(4 * qi, 4 * qi + 4)
                         for nh in range(2))
            while projq:
                proj_unit(*projq.pop(0))
            while inject:
                inject.pop(0)()
            while inject2:
                inject2.pop(0)()

    nc.finalize()
    return nc


def _host_inputs(x, w_attn, b_attn, w_proj, p_param):
    """Build the 8 per-core input dicts."""
    bf16 = ml_dtypes.bfloat16
    ident = np.concatenate([np.eye(64, dtype=np.float32)] * 2, axis=0)
    xx = np.arange(P, dtype=np.int64)[:, None]
    yy = np.arange(512, dtype=np.int64)[None, :]
    masks = np.concatenate(
        [(yy - xx - P * j >= 0).astype(np.float32) for j in range(4)], axis=1)
    onesc = np.ones((P, NK), dtype=np.float32)
    cst = np.concatenate([masks, ident, onesc], axis=1).astype(bf16)
    onesr = np.ones((1, 64), dtype=np.float32)

    xts = [np.ascontiguousarray(x[b].T).astype(bf16) for b in range(B)]
    in_maps = []
    for core in range(8):
        b, hg = divmod(core, 4)
        cs = slice(hg * CL, (hg + 1) * CL)
        csC = slice(C + hg * CL, C + (hg + 1) * CL)
        cs2C = slice(2 * C + hg * CL, 2 * C + (hg + 1) * CL)
        in_maps.append({
            "xt": xts[b],
            "wq": np.ascontiguousarray(w_attn[:, cs]).astype(bf16),
            "wk": np.ascontiguousarray(w_attn[:, csC]).astype(bf16),
            "wv": np.ascontiguousarray(w_attn[:, cs2C]).astype(bf16),
            "wp": np.ascontiguousarray(w_proj[cs, :]).astype(bf16),
            "bps": np.ascontiguousarray(np.concatenate([
                b_attn[cs].reshape(2, P).T,
                b_attn[csC].reshape(2, P).T,
                (b_attn[cs2C] + SHIFT).reshape(2, P).T,
                p_param[cs].reshape(2, P).T,
            ], axis=1).astype(np.float32)),
            "cst": cst,
            "onesr": onesr,
        })
    return in_maps


def kernel(x, w_attn, b_attn, w_proj, b_proj, p_param, _trace=False):
    x = np.asarray(x, dtype=np.float32)
    w_attn = np.asarray(w_attn, dtype=np.float32)
    b_attn = np.asarray(b_attn, dtype=np.float32)
    w_proj = np.asarray(w_proj, dtype=np.float32)
    b_proj = np.asarray(b_proj, dtype=np.float32)
    p_param = np.asarray(p_param, dtype=np.float32)

    # p == 1 admits a cheaper final transform (no per-tile ln/exp)
    p_eff = np.sign(np.sign(p_param) + 0.5) * np.clip(np.abs(p_param),
                                                      P_MIN, P_MAX)
    fast_p1 = bool(np.all(p_eff == 1.0))

    key = ("nc", fast_p1)
    if key not in _CACHE:
        _CACHE[key] = _build(fast_p1)
    nc = _CACHE[key]

    in_maps = _host_inputs(x, w_attn, b_attn, w_proj, p_param)
    res = run_bass_kernel_spmd(nc, in_maps, core_ids=list(range(8)),
                               trace=_trace)
    _CACHE["last_result"] = res

    out = np.zeros((B, T, C), dtype=np.float32)
    for core in range(8):
        b = core // 4
        out[b] += res.results[core]["out_p"]
    out += b_proj[None, None, :]
    return out


if __name__ == "__main__":
    rng = np.random.default_rng(0)
    ins = {
        "x": rng.standard_normal((B, T, C), dtype=np.float32),
        "w_attn": (rng.standard_normal((C, 3 * C), dtype=np.float32) * 0.02),
        "b_attn": np.zeros(3 * C, np.float32),
        "w_proj": (rng.standard_normal((C, C), dtype=np.float32) * 0.02),
        "b_proj": np.zeros(C, np.float32),
        "p_param": np.ones(C, np.float32),
    }
    out = kernel(**ins)
    print("ran, out shape", out.shape, "finite:", np.isfinite(out).all())


# revision 18
# speedup vs baseline: 1.0794x; 1.0794x over previous
"""Trainium2 Bass kernel for nn_CausalGemAttention.

Reference computation (B=2, T=2048, C=1024, H=16, d=64):
    qkv = x @ w_attn + b_attn ; q,k,v = split(qkv)
    p = sign(sign(p_param)+0.5) * clamp(|p_param|, 1e-4, 1e3)
    vc = clip(|v + 5|, 1e-10); z = p*ln(vc); zmax = max_T(z); v' = exp(z - zmax)
    att = causal_softmax(q k^T / sqrt(d)); mean = att @ v'
    y = exp((zmax + ln(mean)) / p) - 5 ; out = y @ w_proj + b_proj

Sharding: 8 cores = 2 (batch) x 4 (head groups of 4 heads / 256 channels).
Each core computes qkv for its head group (contraction over full C), local
attention, and a partial projection (w_proj rows of its channels); host sums
the 4 partials per batch and adds b_proj.

Matmul operands are bf16 with fp32 PSUM accumulation.  v' is centered per
channel before the PV matmul (mean = num''/den + cmid, v'' = v' - cmid)
to keep bf16 rounding out of the softmax average.  When p == 1 the final
transform is y = ezp*mean - 5 without per-tile ln/exp.

Schedule (v2): single pool region, fully pipelined.
  - inputs stream per 512-column t-block; v matmuls start as soon as block 0
    lands; ln/max/min per block; zmax folded into one Exp (scale=p).
  - scores use the stacked two-head k tile as stationary with per-head
    zero-padded q as moving operand (no padded k buffer needed).
  - the attention phase is paced by ScalarE exp; remaining PE work (q/k for
    the second head pair, v transposes, projection tiles) is injected one
    item per softmax pair-slot so the PE never bursts ahead of ACT.
  - post-chains read PV PSUM directly (reciprocal + scalar_tensor_tensor),
    projection PSUM is drained on DVE, output DMA on the idle Sync queue.
"""

import sys
sys.path.insert(0, "/opt/trn_rl_repo")

import numpy as np
import ml_dtypes

import concourse.bacc as bacc
import concourse.tile as tile
from concourse import mybir
from concourse.bass_utils import run_bass_kernel_spmd

F32 = mybir.dt.float32
F32R = mybir.dt.float32r
F16 = mybir.dt.float16
BF16 = mybir.dt.bfloat16
AF = mybir.ActivationFunctionType
ALU = mybir.AluOpType
AX = mybir.AxisListType

B, T, C, H, D = 2, 2048, 1024, 16, 64
P = 128
CL = 256            # channels per core (4 heads x 64)
KC = C // P         # 8 contraction chunks for qkv
NQ = T // 512       # 4 query blocks of 512
NK = T // P         # 16 key tiles of 128
SHIFT = 5.0
P_MIN, P_MAX, V_MIN = 1e-4, 1e3, 1e-10
SM_SCALE = 1.0 / 8.0  # 1/sqrt(64)

_CACHE = {}


def _build(fast_p1):
    nc = bacc.Bacc("TRN2", target_bir_lowering=False, debug=False)

    xt_d = nc.dram_tensor("xt", [C, T], BF16, kind="ExternalInput")
    wq_d = nc.dram_tensor("wq", [C, CL], BF16, kind="ExternalInput")
    wk_d = nc.dram_tensor("wk", [C, CL], BF16, kind="ExternalInput")
    wv_d = nc.dram_tensor("wv", [C, CL], BF16, kind="ExternalInput")
    wp_d = nc.dram_tensor("wp", [CL, C], BF16, kind="ExternalInput")
    # bps = [bq(2) | bk(2) | bv5(2) | pp(2)] per chunk, fp32
    bps_d = nc.dram_tensor("bps", [P, 8], F32, kind="ExternalInput")
    # cst = [masks(4x512) | ident(64) | onesc(16)] packed along free dim
    cst_d = nc.dram_tensor("cst", [P, 2128], BF16, kind="ExternalInput")
    or_d = nc.dram_tensor("onesr", [1, 64], F32R, kind="ExternalInput")
    out_d = nc.dram_tensor("out_p", [T, C], F16, kind="ExternalOutput")

    with tile.TileContext(nc) as tc:
        with (
            tc.tile_pool(name="consts", bufs=1) as cp,
            tc.tile_pool(name="big", bufs=1) as bg,
            tc.tile_pool(name="att", bufs=5) as att,
            tc.tile_pool(name="small", bufs=3) as sm,
            tc.tile_pool(name="outp", bufs=3) as op_,
            tc.tile_pool(name="psS", bufs=2, space="PSUM") as psS,
            tc.tile_pool(name="psV", bufs=2, space="PSUM") as psV,
            tc.tile_pool(name="psX", bufs=2, space="PSUM") as psX,
        ):
            # ------------- static tiles -------------
            cst = cp.tile([P, 2128], BF16)
            onesr = cp.tile([1, 64], F32R)
            bps = cp.tile([P, 8], F32)
            xt_sb = bg.tile([P, KC, T], BF16)
            wq_sb = bg.tile([P, KC, CL], BF16)
            wk_sb = bg.tile([P, KC, CL], BF16)
            wv_sb = bg.tile([P, KC, CL], BF16)
            wp_sb = bg.tile([P, 2, C], BF16)
            qT = bg.tile([P, 2, T], BF16)     # q^T: [c%128, c//128, t]
            kT = bg.tile([P, 2, T], BF16)     # stacked: rows 0:64 head even,
            #                                   rows 64:128 head odd (per m)
            qTp = bg.tile([P, 4, T], BF16)    # per-head q, zero-padded to 128
            vnat = bg.tile([P, 4, NK, 65], BF16)  # [tk%128, head, tk//128, d|1]
            yT = bg.tile([P, 2, T], BF16)
            vT = bg.tile([P, 2, T], F32)      # |v+5| -> ln -> v'
            vpT = bg.tile([P, 2, T], BF16)    # centered v'' (bf16)

            ident = cst[:, 2048:2112]
            bq_sb = bps[:, 0:2]
            bk_sb = bps[:, 2:4]
            bv5_sb = bps[:, 4:6]
            pp_sb = bps[:, 6:8]

            # ------------- input DMA (ordered for pipelining) -------------
            nc.gpsimd.dma_start(bps[:], bps_d[:])
            nc.sync.dma_start(wv_sb[:], wv_d[:].rearrange("(a p) m -> p a m", p=P))
            # xt block 0 in two chunk-halves so v matmuls start asap
            nc.sync.dma_start(
                xt_sb[:, 0:4, 0:512],
                xt_d[0:512, 0:512].rearrange("(a p) m -> p a m", p=P))
            nc.sync.dma_start(
                xt_sb[:, 4:8, 0:512],
                xt_d[512:1024, 0:512].rearrange("(a p) m -> p a m", p=P))
            for b_ in range(1, NQ):
                bsl = slice(b_ * 512, (b_ + 1) * 512)
                nc.sync.dma_start(
                    xt_sb[:, :, bsl],
                    xt_d[:, bsl].rearrange("(a p) m -> p a m", p=P))
            nc.sync.dma_start(wq_sb[:], wq_d[:].rearrange("(a p) m -> p a m", p=P))
            nc.sync.dma_start(wk_sb[:], wk_d[:].rearrange("(a p) m -> p a m", p=P))
            nc.sync.dma_start(cst[:], cst_d[:])
            nc.sync.dma_start(onesr[:], or_d[:])
            nc.sync.dma_start(wp_sb[:], wp_d[:].rearrange("(c p) n -> p c n", p=P))
            for h in range(4):
                nc.sync.dma_start(vnat[:, h, :, 64], cst_d[:, 2112:2128])

            # qTp zero padding: even heads pad rows 64:128, odd pad 0:64
            nc.gpsimd.memset(qTp[64:128, 0, :], 0.0)
            nc.gpsimd.memset(qTp[0:64, 1, :], 0.0)
            nc.gpsimd.memset(qTp[64:128, 2, :], 0.0)
            nc.gpsimd.memset(qTp[0:64, 3, :], 0.0)

            # ------------- p transform (tiny) -------------
            sgn = cp.tile([P, 2], F32)
            ab = cp.tile([P, 2], F32)
            p_sb = cp.tile([P, 2], F32)
            # allcp packs [ip | zmaxp | cmid | ezp | ecp5] x 2 chunks
            allcp = cp.tile([P, 5, 2], F32)
            ip_sb = allcp[:, 0, :]
            zmaxp = allcp[:, 1, :]
            cmid = allcp[:, 2, :]
            ezp = allcp[:, 3, :]
            ecp5 = allcp[:, 4, :]
            nc.scalar.activation(sgn[:], pp_sb[:], AF.Sign)
            nc.vector.tensor_scalar_add(sgn[:], sgn[:], 0.5)
            nc.scalar.activation(sgn[:], sgn[:], AF.Sign)
            nc.scalar.activation(ab[:], pp_sb[:], AF.Abs)
            nc.vector.tensor_scalar(ab[:], ab[:], float(P_MIN), float(P_MAX),
                                    ALU.max, ALU.min)
            nc.vector.tensor_tensor(p_sb[:], sgn[:], ab[:], ALU.mult)
            nc.vector.reciprocal(ip_sb[:], p_sb[:])

            if not fast_p1:
                blmax = cp.tile([P, 2, NQ], F32)
                blmin = cp.tile([P, 2, NQ], F32)
            mxa = cp.tile([P, 2], F32)
            mna = cp.tile([P, 2], F32)
            if not fast_p1:
                negzmax = cp.tile([P, 2], F32)
                zmin_g = cp.tile([P, 2], F32)
            # allh: per-head [64,1] base-0 views of allcp, [64, const, head]
            allh = cp.tile([64, 5, 4], F32)
            iph = allh[:, 0, :]
            zmh = allh[:, 1, :]
            cmh = allh[:, 2, :]
            eph = allh[:, 3, :]
            ech = allh[:, 4, :]

            # ------------- helpers for phase A -------------
            def derived_consts():
                # from mxa/mna (max/min of ln vc over the chosen range):
                # z = p*lnv; zmax = max(p*maxln, p*minln), zmin = min(...)
                nc.vector.tensor_tensor(mxa[:], p_sb[:], mxa[:], ALU.mult)
                nc.vector.tensor_tensor(mna[:], p_sb[:], mna[:], ALU.mult)
                nc.vector.tensor_tensor(zmin_g[:], mxa[:], mna[:], ALU.min)
                nc.vector.tensor_tensor(negzmax[:], mxa[:], mna[:], ALU.max)
                nc.vector.tensor_scalar_mul(negzmax[:], negzmax[:], -1.0)
                # cmid = 0.5*(1 + exp(zmin - zmax))
                for m in range(2):
                    nc.scalar.activation(cmid[:, m:m + 1], zmin_g[:, m:m + 1],
                                         AF.Exp, bias=negzmax[:, m:m + 1])
                nc.vector.tensor_scalar(cmid[:], cmid[:], 1.0, 0.5,
                                        ALU.add, ALU.mult)
                # ezp = exp(zmax); ecp5 = ezp*cmid - 5
                nc.scalar.activation(ezp[:], negzmax[:], AF.Exp, scale=-1.0)
                nc.vector.scalar_tensor_tensor(ecp5[:], ezp[:], 0.0, cmid[:],
                                               ALU.bypass, ALU.mult)
                nc.vector.tensor_scalar_add(ecp5[:], ecp5[:], -SHIFT)
                # zmaxp = zmax * ip (general-p path)
                nc.vector.scalar_tensor_tensor(zmaxp[:], negzmax[:], -1.0,
                                               ip_sb[:], ALU.mult, ALU.mult)
                # per-head constants at partition base 0
                nc.sync.dma_start(allh[:, :, 0::2], allcp[0:64, :, :])
                nc.sync.dma_start(allh[:, :, 1::2], allcp[64:128, :, :])

            def v_exp_block(b_):
                # v' = exp(p*lnv - zmax); v'' = v' - cmid
                bsl = slice(b_ * 512, (b_ + 1) * 512)
                for m in range(2):
                    nc.scalar.activation(vT[:, m, bsl], vT[:, m, bsl], AF.Exp,
                                         scale=p_sb[:, m:m + 1],
                                         bias=negzmax[:, m:m + 1])
                    nc.vector.tensor_scalar_sub(vpT[:, m, bsl], vT[:, m, bsl],
                                                cmid[:, m:m + 1])

            def trp_group(h, half):
                # transpose v''^T [d, tk] -> vnat [tk, d], 8 k-tiles
                base, ch = 64 * (h % 2), h // 2
                trp = psX.tile([P, 512], BF16, tag="x", name="trp")
                for j in range(8):
                    kt = half * 8 + j
                    nc.tensor.transpose(
                        trp[:, j * 64:(j + 1) * 64],
                        vpT[base:base + 64, ch, kt * P:(kt + 1) * P],
                        cst[base:base + 64, 2048:2112])
                nc.vector.tensor_copy(
                    vnat[:, h, half * 8:(half + 1) * 8, 0:64],
                    trp[:].rearrange("p (a b) -> p a b", a=8))

            def qk_seq(kind, m, nt, on_act):
                wsb = wq_sb if kind == "q" else wk_sb
                dst = qT if kind == "q" else kT
                bsb = bq_sb if kind == "q" else bk_sb
                ps = psX.tile([P, 512], F32, tag="x", name="ev")
                for kc in range(KC):
                    nc.tensor.matmul(
                        ps[:], wsb[:, kc, m * P:(m + 1) * P],
                        xt_sb[:, kc, nt * 512:(nt + 1) * 512],
                        start=(kc == 0), stop=(kc == KC - 1))
                tsl = slice(nt * 512, (nt + 1) * 512)
                if on_act:
                    nc.scalar.activation(dst[:, m, tsl], ps[:], AF.Identity,
                                         bias=bsb[:, m:m + 1])
                else:
                    nc.vector.tensor_scalar_add(dst[:, m, tsl], ps[:],
                                                bsb[:, m:m + 1])

            def qpads(m):
                h0, h1 = 2 * m, 2 * m + 1
                nc.gpsimd.dma_start(qTp[0:64, h0, :], qT[0:64, m, :])
                nc.gpsimd.dma_start(qTp[64:128, h1, :], qT[64:128, m, :])

            # ------------- per-block v/q/k matmuls + streamed transform ------
            # fast_p1 (p == 1): the log-power transform collapses to plain
            # per-channel scaling: v' = vc / mref, ezp = mref, with
            # mref/cmid taken from block 0 only (algebraically exact for any
            # per-channel reference; block-0 max keeps v' bounded near 1).
            # No Ln/Exp at all, so ACT stays in one table set.  General p
            # keeps the exact log-domain path with a global max.
            rmx = cp.tile([P, 2], F32)
            for b_ in range(NQ):
                bsl = slice(b_ * 512, (b_ + 1) * 512)
                for m in range(2):
                    ps = psX.tile([P, 512], F32, tag="x", name="ev")
                    for kc in range(KC):
                        nc.tensor.matmul(
                            ps[:], wv_sb[:, kc, m * P:(m + 1) * P],
                            xt_sb[:, kc, bsl],
                            start=(kc == 0), stop=(kc == KC - 1))
                    nc.scalar.activation(vT[:, m, bsl], ps[:], AF.Abs,
                                         bias=bv5_sb[:, m:m + 1])
                if fast_p1:
                    if b_ == 0:
                        nc.vector.tensor_reduce(mxa[:, 0:1], vT[:, 0, bsl],
                                                AX.X, op=ALU.max)
                        nc.vector.tensor_reduce(mxa[:, 1:2], vT[:, 1, bsl],
                                                AX.X, op=ALU.max)
                        nc.vector.tensor_reduce(mna[:, 0:1], vT[:, 0, bsl],
                                                AX.X, op=ALU.min)
                        nc.vector.tensor_reduce(mna[:, 1:2], vT[:, 1, bsl],
                                                AX.X, op=ALU.min)
                        nc.vector.reciprocal(rmx[:], mxa[:])
                        # cmid = 0.5*(1 + mn/mx); ezp = mx; ecp5 = mx*cmid-5
                        nc.vector.tensor_tensor(cmid[:], mna[:], rmx[:],
                                                ALU.mult)
                        nc.vector.tensor_scalar(cmid[:], cmid[:], 1.0, 0.5,
                                                ALU.add, ALU.mult)
                        nc.vector.tensor_copy(ezp[:], mxa[:])
                        nc.vector.tensor_tensor(ecp5[:], mxa[:], cmid[:],
                                                ALU.mult)
                        nc.vector.tensor_scalar_add(ecp5[:], ecp5[:], -SHIFT)
                        # ip/zmaxp unused on this path; keep them defined
                        nc.vector.tensor_copy(zmaxp[:], mxa[:])
                        nc.sync.dma_start(allh[:, :, 0::2], allcp[0:64, :, :])
                        nc.sync.dma_start(allh[:, :, 1::2],
                                          allcp[64:128, :, :])
                    # v'' = vc/mref - cmid in one DVE op
                    for m in range(2):
                        nc.vector.tensor_scalar(vpT[:, m, bsl], vT[:, m, bsl],
                                                rmx[:, m:m + 1],
                                                cmid[:, m:m + 1],
                                                ALU.mult, ALU.subtract)
                else:
                    for m in range(2):
                        nc.vector.tensor_scalar_max(vT[:, m, bsl],
                                                    vT[:, m, bsl],
                                                    float(V_MIN))
                    for m in range(2):
                        nc.scalar.activation(vT[:, m, bsl], vT[:, m, bsl],
                                             AF.Ln)
                    for m in range(2):
                        nc.vector.tensor_reduce(blmax[:, m, b_:b_ + 1],
                                                vT[:, m, bsl], AX.X,
                                                op=ALU.max)
                        nc.vector.tensor_reduce(blmin[:, m, b_:b_ + 1],
                                                vT[:, m, bsl], AX.X,
                                                op=ALU.min)
                # q/k for head pair 0 on this t-block (only needs xt block b)
                qk_seq("q", 0, b_, True)
                qk_seq("k", 0, b_, True)
                nc.gpsimd.dma_start(qTp[0:64, 0, bsl], qT[0:64, 0, bsl])
                nc.gpsimd.dma_start(qTp[64:128, 1, bsl], qT[64:128, 0, bsl])
                if fast_p1 and b_ == 1:
                    trp_group(0, 0)
                    trp_group(1, 0)

            if not fast_p1:
                nc.vector.tensor_reduce(mxa[:, 0:1], blmax[:, 0, :], AX.X,
                                        op=ALU.max)
                nc.vector.tensor_reduce(mxa[:, 1:2], blmax[:, 1, :], AX.X,
                                        op=ALU.max)
                nc.vector.tensor_reduce(mna[:, 0:1], blmin[:, 0, :], AX.X,
                                        op=ALU.min)
                nc.vector.tensor_reduce(mna[:, 1:2], blmin[:, 1, :], AX.X,
                                        op=ALU.min)
                derived_consts()
                for b_ in range(NQ):
                    v_exp_block(b_)
                trp_group(0, 0)
                trp_group(1, 0)

            # ------------- attention -------------
            pending = []   # deferred post-chains (emitted mid next q-block)
            projq = []     # projection tiles ready to emit

            po_ref = {}

            def proj_unit(tq, nh):
                if nh == 0:
                    po_ref[tq] = op_.tile([P, C], F16, tag="po", name="po")
                po = po_ref[tq]
                pj = psX.tile([P, 512], F32, tag="x", name="pj")
                for c in range(2):
                    nc.tensor.matmul(
                        pj[:], yT[:, c, tq * P:(tq + 1) * P],
                        wp_sb[:, c, nh * 512:(nh + 1) * 512],
                        start=(c == 0), stop=(c == 1))
                nc.vector.tensor_copy(po[:, nh * 512:(nh + 1) * 512], pj[:])
                if nh == 1:
                    nc.sync.dma_start(out_d[tq * P:(tq + 1) * P, :], po[:])
                    del po_ref[tq]

            def post_chain(pv_t, h, qi_):
                # mean'' = num''/den ; y = ezp*mean'' + (ezp*cmid - 5)
                dcp = sm.tile([1, 512], F32, tag="dcp", name="dcp")
                nc.vector.tensor_copy(dcp[:], pv_t[64:65, :])
                rdf = sm.tile([1, 512], F32, tag="rdf", name="rdf")
                nc.vector.reciprocal_approx_fast(rdf[:], dcp[:])
                bcs = sm.tile([64, 512], F32, tag="bc", name="bcs")
                nc.gpsimd.partition_broadcast(bcs[:], rdf[:])
                yh = sm.tile([64, 512], BF16, tag="yh", name="yh")
                if fast_p1:
                    y1 = sm.tile([64, 512], F32, tag="y1", name="y1")
                    nc.vector.scalar_tensor_tensor(
                        y1[:], pv_t[0:64, :], eph[:, h:h + 1], bcs[:],
                        ALU.mult, ALU.mult)
                    nc.vector.tensor_scalar_add(yh[:], y1[:], ech[:, h:h + 1])
                else:
                    me = sm.tile([64, 512], F32, tag="y1", name="me")
                    nc.vector.tensor_tensor(me[:], pv_t[0:64, :], bcs[:],
                                            ALU.mult)
                    nc.vector.tensor_scalar_add(me[:], me[:], cmh[:, h:h + 1])
                    nc.scalar.activation(me[:], me[:], AF.Ln)
                    nc.scalar.activation(yh[:], me[:], AF.Exp,
                                         scale=iph[:, h:h + 1],
                                         bias=zmh[:, h:h + 1])
                    nc.vector.tensor_scalar_add(yh[:], yh[:], -SHIFT)
                base, ch = 64 * (h % 2), h // 2
                nc.gpsimd.dma_start(
                    yT[base:base + 64, ch, qi_ * 512:(qi_ + 1) * 512], yh[:])

            # PE work injected one item per softmax pair-slot during hp=0
            inject = []
            inject.append(lambda: trp_group(0, 1))
            inject.append(lambda: trp_group(1, 1))
            for nt in range(NQ):
                inject.append(lambda nt=nt: qk_seq("q", 1, nt, False))
            inject.append(lambda: qpads(1))
            for nt in range(NQ):
                inject.append(lambda nt=nt: qk_seq("k", 1, nt, False))
            inject.append(lambda: trp_group(2, 0))
            inject.append(lambda: trp_group(3, 0))
            # second head pair's late transposes go into hp=1's idle slots
            inject2 = [lambda: trp_group(2, 1), lambda: trp_group(3, 1)]

            def flush_pending():
                for fn in pending:
                    fn()
                pending.clear()

            def slot_work(hp):
                if hp == 0:
                    if inject:
                        inject.pop(0)()
                else:
                    if inject2:
                        inject2.pop(0)()
                    elif projq:
                        proj_unit(*projq.pop(0))

            def mk(pv_t, h, qi_, push_proj):
                def fn():
                    post_chain(pv_t, h, qi_)
                    if push_proj:
                        projq.extend((tq, nh) for tq in
                                     range(4 * qi_, 4 * qi_ + 4)
                                     for nh in range(2))
                return fn

            def scores_mm(s_t, h, hp, qi, a):
                for half in range(2):
                    kt = 2 * a + half
                    off = P * max(kt - 4 * qi, 0)
                    ksl = slice(kt * P, (kt + 1) * P)
                    qsub = slice(qi * 512 + off, (qi + 1) * 512)
                    nc.tensor.matmul(
                        s_t[:, half * 512 + off:(half + 1) * 512],
                        kT[:, hp, ksl], qTp[:, h, qsub],
                        start=True, stop=True, skip_group_check=True)

            def exp_mask(s_t, qi, a):
                j0 = 2 * a - 4 * qi
                off0 = P * max(j0, 0)
                pt = att.tile([P, 1024], BF16, tag="pT", name="pt")
                nc.scalar.activation(pt[:, off0:1024], s_t[:, off0:1024],
                                     AF.Exp, scale=SM_SCALE)
                if j0 >= 0:   # diagonal band: mask pair
                    nc.vector.tensor_mul(
                        pt[:, off0:1024], pt[:, off0:1024],
                        cst[:, j0 * 512 + off0:(j0 + 2) * 512])
                return pt

            def pv_mm(pv_t, h, qi, a, pt, last):
                for half in range(2):
                    kt = 2 * a + half
                    o_ = P * max(kt - 4 * qi, 0)
                    nc.tensor.matmul(
                        pv_t[:, o_:512], vnat[:, h, kt, :],
                        pt[:, half * 512 + o_:(half + 1) * 512],
                        start=(kt == 0), stop=(last and half == 1),
                        skip_group_check=True)

            for hp in range(2):
                h0, h1 = 2 * hp, 2 * hp + 1
                nq_pair = NQ - 1 if hp == 1 else NQ
                for qi in range(nq_pair):
                    npair = 2 * (qi + 1)       # kt pairs (kt = 2a, 2a+1)
                    pv = [psV.tile([65, 512], F32, tag="pv", name=f"pv{_i}")
                          for _i in range(2)]
                    prev = None
                    for a in range(npair):
                        ptile = []
                        s_ps = [psS.tile([P, 1024], F32, tag="s",
                                         name=f"s{_i}") for _i in range(2)]
                        for i, h in enumerate((h0, h1)):
                            scores_mm(s_ps[i], h, hp, qi, a)
                        slot_work(hp)
                        for i in range(2):
                            ptile.append(exp_mask(s_ps[i], qi, a))
                        if prev is not None:
                            pa_, pp0, pp1 = prev
                            for i, ppt in enumerate((pp0, pp1)):
                                pv_mm(pv[i], (h0, h1)[i], qi, pa_, ppt, False)
                        if a == 0 and pending:
                            flush_pending()
                        if hp == 1 and projq:
                            proj_unit(*projq.pop(0))
                        prev = (a, ptile[0], ptile[1])
                    pa_, pp0, pp1 = prev
                    for i, ppt in enumerate((pp0, pp1)):
                        pv_mm(pv[i], (h0, h1)[i], qi, pa_, ppt, True)
                    pending.append(mk(pv[0], h0, qi, False))
                    pending.append(mk(pv[1], h1, qi, hp == 1))

            # last q-block of the second head pair: process heads one after
            # the other so h2's post-chain overlaps h3's scores/PV on PE
            qi = NQ - 1
            npair = 2 * (qi + 1)
            pv = [psV.tile([65, 512], F32, tag="pv", name=f"pvl{_i}")
                  for _i in range(2)]
            for i, h in ((0, 2), (1, 3)):
                prevh = None
                for a in range(npair):
                    s_ = psS.tile([P, 1024], F32, tag="s", name="sl")
                    scores_mm(s_, h, 1, qi, a)
                    if i == 0 and a == 0 and pending:
                        flush_pending()
                    if projq:
                        proj_unit(*projq.pop(0))
                    pt = exp_mask(s_, qi, a)
                    if prevh is not None:
                        pv_mm(pv[i], h, qi, prevh[0], prevh[1], False)
                    prevh = (a, pt)
                pv_mm(pv[i], h, qi, prevh[0], prevh[1], True)
                post_chain(pv[i], h, qi)
            projq.extend((tq, nh) for tq in range(4 * qi, 4 * qi + 4)
                         for nh in range(2))
            while projq:
                proj_unit(*projq.pop(0))
            while inject:
                inject.pop(0)()
            while inject2:
                inject2.pop(0)()

    nc.finalize()
    return nc


def _host_inputs(x, w_attn, b_attn, w_proj, p_param):
    """Build the 8 per-core input dicts."""
    bf16 = ml_dtypes.bfloat16
    ident = np.concatenate([np.eye(64, dtype=np.float32)] * 2, axis=0)
    xx = np.arange(P, dtype=np.int64)[:, None]
    yy = np.arange(512, dtype=np.int64)[None, :]
    masks = np.concatenate(
        [(yy - xx - P * j >= 0).astype(np.float32) for j in range(4)], axis=1)
    onesc = np.ones((P, NK), dtype=np.float32)
    cst = np.concatenate([masks, ident, onesc], axis=1).astype(bf16)
    onesr = np.ones((1, 64), dtype=np.float32)

    xts = [np.ascontiguousarray(x[b].T).astype(bf16) for b in range(B)]
    in_maps = []
    for core in range(8):
        b, hg = divmod(core, 4)
        cs = slice(hg * CL, (hg + 1) * CL)
        csC = slice(C + hg * CL, C + (hg + 1) * CL)
        cs2C = slice(2 * C + hg * CL, 2 * C + (hg + 1) * CL)
        in_maps.append({
            "xt": xts[b],
            "wq": np.ascontiguousarray(w_attn[:, cs]).astype(bf16),
            "wk": np.ascontiguousarray(w_attn[:, csC]).astype(bf16),
            "wv": np.ascontiguousarray(w_attn[:, cs2C]).astype(bf16),
            "wp": np.ascontiguousarray(w_proj[cs, :]).astype(bf16),
            "bps": np.ascontiguousarray(np.concatenate([
                b_attn[cs].reshape(2, P).T,
                b_attn[csC].reshape(2, P).T,
                (b_attn[cs2C] + SHIFT).reshape(2, P).T,
                p_param[cs].reshape(2, P).T,
            ], axis=1).astype(np.float32)),
            "cst": cst,
            "onesr": onesr,
        })
    return in_maps


def kernel(x, w_attn, b_attn, w_proj, b_proj, p_param, _trace=False):
    x = np.asarray(x, dtype=np.float32)
    w_attn = np.asarray(w_attn, dtype=np.float32)
    b_attn = np.asarray(b_attn, dtype=np.float32)
    w_proj = np.asarray(w_proj, dtype=np.float32)
    b_proj = np.asarray(b_proj, dtype=np.float32)
    p_param = np.asarray(p_param, dtype=np.float32)

    # p == 1 admits a cheaper final transform (no per-tile ln/exp)
    p_eff = np.sign(np.sign(p_param) + 0.5) * np.clip(np.abs(p_param),
                                                      P_MIN, P_MAX)
    fast_p1 = bool(np.all(p_eff == 1.0))

    key = ("nc", fast_p1)
    if key not in _CACHE:
        _CACHE[key] = _build(fast_p1)
    nc = _CACHE[key]

    in_maps = _host_inputs(x, w_attn, b_attn, w_proj, p_param)
    res = run_bass_kernel_spmd(nc, in_maps, core_ids=list(range(8)),
                               trace=_trace)
    _CACHE["last_result"] = res

    out = np.zeros((B, T, C), dtype=np.float32)
    for core in range(8):
        b = core // 4
        out[b] += res.results[core]["out_p"]
    out += b_proj[None, None, :]
    return out


if __name__ == "__main__":
    rng = np.random.default_rng(0)
    ins = {
        "x": rng.standard_normal((B, T, C), dtype=np.float32),
        "w_attn": (rng.standard_normal((C, 3 * C), dtype=np.float32) * 0.02),
        "b_attn": np.zeros(3 * C, np.float32),
        "w_proj": (rng.standard_normal((C, C), dtype=np.float32) * 0.02),
        "b_proj": np.zeros(C, np.float32),
        "p_param": np.ones(C, np.float32),
    }
    out = kernel(**ins)
    print("ran, out shape", out.shape, "finite:", np.isfinite(out).all())
